# revision 1
# baseline (speedup 1.0000x reference)
"""Trainium2 Bass kernel for nn_ConvocationV3 (dense_cnn).

Pipeline per sample (B=32, C=384, H=W=54, K=3):
  value = conv1x1(x, w_v) ; qk = pool3x3(conv1x1(x, w_qk)) = conv1x1(pool3x3(x), w_qk)
  h = gelu(conv1x1(qk, w_kg1)) ; kernels = conv1x1(h, w_kg2)
  kernels -= sigmoid(beta)/9 * sum_taps(kernels)
  out = depthwise3x3(value, kernels)  (per-sample, per-channel kernels)
  y = conv1x1(out, w_proj)

Sharding: data-parallel over batch, 4 samples per core on 8 cores.
"""

import numpy as np
import ml_dtypes

import concourse.bass as bass
import concourse.bacc as bacc
import concourse.mybir as mybir
import concourse.tile as tile
from concourse.bass_utils import run_bass_kernel_spmd

F32 = mybir.dt.float32
F32R = mybir.dt.float32r
BF16 = mybir.dt.bfloat16
AX = mybir.AxisListType
ALU = mybir.AluOpType
ACTF = mybir.ActivationFunctionType

B_LOC = 4          # samples per core
CT = 3             # channel tiles (384 = 3*128)
P = 128
HW = 2916          # 54*54
PW = 56            # padded width/height
BIG = 972          # dma/act chunk (18 rows of 54)
CH = 486           # matmul free chunk (9 rows of 54)
DQ = 96

# taps: t = 3*i + j, flat offset (i-1)*56 + (j-1) on the padded plane.
# PE taps: diag matmuls into psum. DVE taps: contiguous flat-shift stt (bf16 2x);
# odd offsets read vpadB (one-element-shifted copy) to stay 4B-aligned.
PE_TAPS = [1, 4, 7, 0]
DVE_TAPS = [2, 3, 5, 6, 8]
VPAD_N = 3200      # vpad tile: 2 lead pad elems + 56*56 grid + tail slack


def build_program():
    nc = bacc.Bacc("TRN2", target_bir_lowering=False, debug=False)

    x_d = nc.dram_tensor("x", [B_LOC, CT, P, HW], BF16, kind="ExternalInput").ap()
    wv_d = nc.dram_tensor("wv", [P, CT, 384], BF16, kind="ExternalInput").ap()
    wproj_d = nc.dram_tensor("wproj", [P, CT, 384], BF16, kind="ExternalInput").ap()
    wqk_d = nc.dram_tensor("wqk", [P, CT, 384], F32, kind="ExternalInput").ap()
    wkg1_d = nc.dram_tensor("wkg1", [P, CT, DQ], F32, kind="ExternalInput").ap()
    wkg2e_d = nc.dram_tensor("wkg2e", [DQ + 1, 384], F32, kind="ExternalInput").ap()
    bv_d = nc.dram_tensor("bv", [P, CT], F32, kind="ExternalInput").ap()
    bqk_d = nc.dram_tensor("bqk", [P, CT], F32, kind="ExternalInput").ap()
    bkg1_d = nc.dram_tensor("bkg1", [DQ, 1], F32, kind="ExternalInput").ap()
    bproj_d = nc.dram_tensor("bproj", [P, CT], F32, kind="ExternalInput").ap()
    fac9_d = nc.dram_tensor("fac9", [P, CT], F32, kind="ExternalInput").ap()
    eye_d = nc.dram_tensor("eye", [P, P], BF16, kind="ExternalInput").ap()

    y_d = nc.dram_tensor("y", [B_LOC, CT, P, HW], F32, kind="ExternalOutput").ap()

    with tile.TileContext(nc) as tc:
        with (
            tc.tile_pool(name="const", bufs=1) as cpool,
            tc.tile_pool(name="xch", bufs=4) as xpool,
            tc.tile_pool(name="vpad", bufs=2) as vppool,
            tc.tile_pool(name="dw", bufs=2) as dwpool,
            tc.tile_pool(name="ych", bufs=4) as ypool,
            tc.tile_pool(name="small", bufs=2) as spool,
            tc.tile_pool(name="mm", bufs=3, space="PSUM") as mmpool,
            tc.tile_pool(name="smallps", bufs=2, space="PSUM") as sppool,
        ):
            # ---- constants ----
            wv = cpool.tile([P, CT, 384], BF16, name="wv_sb")
            wproj = cpool.tile([P, CT, 384], BF16, name="wproj_sb")
            wqk = cpool.tile([P, CT, 384], F32, name="wqk_sb")
            wkg1 = cpool.tile([P, CT, DQ], F32, name="wkg1_sb")
            wkg2e = cpool.tile([DQ + 1, 384], F32, name="wkg2e_sb")
            bv = cpool.tile([P, CT], F32, name="bv_sb")
            bqk = cpool.tile([P, CT], F32, name="bqk_sb")
            bkg1 = cpool.tile([DQ, 1], F32, name="bkg1_sb")
            bproj = cpool.tile([P, CT], F32, name="bproj_sb")
            fac9 = cpool.tile([P, CT], F32, name="fac9_sb")
            eye = cpool.tile([P, P], BF16, name="eye_sb")
            for t_sb, t_dr in [(wv, wv_d), (wproj, wproj_d), (wqk, wqk_d),
                               (wkg1, wkg1_d), (wkg2e, wkg2e_d), (bv, bv_d),
                               (bqk, bqk_d), (bkg1, bkg1_d), (bproj, bproj_d),
                               (fac9, fac9_d), (eye, eye_d)]:
                nc.sync.dma_start(t_sb[:], t_dr[:])

            for b in range(B_LOC):
                # ---- stage A: x load, pooling stage 1, value conv -> vpad ----
                pool1 = spool.tile([P, CT, 54, 3], F32, name=f"pool1_{b}", tag="pool1")
                vpads = []
                vpbs = []
                for ct in range(CT):
                    vp = vppool.tile([P, VPAD_N], BF16, name=f"vpad_{b}_{ct}",
                                     tag=f"vpad{ct}")
                    vpb = vppool.tile([P, VPAD_N], BF16, name=f"vpb_{b}_{ct}",
                                      tag=f"vpb{ct}")
                    vpads.append(vp)
                    vpbs.append(vpb)
                    vpv = vp[:, 2:2 + PW * PW].rearrange("p (h w) -> p h w", h=PW)
                    # zero borders (interior is fully overwritten by ACT)
                    nc.gpsimd.memset(vpv[:, 0:1, :], 0.0)
                    nc.gpsimd.memset(vpv[:, PW - 1:PW, :], 0.0)
                    nc.gpsimd.memset(vpv[:, 1:PW - 1, 0:1], 0.0)
                    nc.gpsimd.memset(vpv[:, 1:PW - 1, PW - 1:PW], 0.0)

                for g in range(3):  # big chunks of 18 rows
                    xch = xpool.tile([P, CT, BIG], BF16, name=f"x_{b}_{g}", tag="xch")
                    nc.sync.dma_start(
                        xch[:], x_d[b, :, :, g * BIG:(g + 1) * BIG].transpose([1, 0, 2]))
                    for kt in range(CT):
                        nc.vector.tensor_reduce(
                            out=pool1[:, kt, g * 18:(g + 1) * 18, :],
                            in_=xch[:, kt].rearrange("p (h wb w) -> p h wb w", wb=3, w=18),
                            axis=AX.X, op=ALU.add)
                    for mt in range(CT):
                        ps = mmpool.tile([P, 2, 512], F32, name=f"vps_{b}_{g}_{mt}", tag="mm")
                        for s in range(2):
                            for kt in range(CT):
                                nc.tensor.matmul(
                                    ps[:, s, :CH],
                                    lhsT=wv[:, kt, mt * P:(mt + 1) * P],
                                    rhs=xch[:, kt, s * CH:(s + 1) * CH],
                                    start=(kt == 0), stop=(kt == CT - 1))
                        # write value (+bias) into padded interior rows, bf16
                        nc.scalar.activation(
                            out=vpads[mt][:, 2:2 + PW * PW].rearrange(
                                "p (h w) -> p h w", h=PW)[
                                :, 1 + g * 18:1 + (g + 1) * 18, 1:55],
                            in_=ps[:, :, :CH],
                            func=ACTF.Identity, bias=bv[:, mt:mt + 1], scale=1.0)

                # ---- stage B: pooling stage 2 -> pooled (sum over 324, /324 in wqk) ----
                pooled = spool.tile([P, CT, 9], F32, name=f"pooled_{b}", tag="pooled")
                for kt in range(CT):
                    nc.vector.tensor_reduce(
                        out=pooled[:, kt].rearrange("p (hb wb) -> p hb wb", hb=3),
                        in_=pool1[:, kt].rearrange("p (hb hs) wb -> p hb wb hs", hb=3),
                        axis=AX.X, op=ALU.add)

                # ---- stage C: qk conv (f32r, tiny) ----
                qk = spool.tile([P, CT, 9], F32, name=f"qk_{b}", tag="qk")
                for mt in range(CT):
                    psq = sppool.tile([P, 9], F32, name=f"qps_{b}_{mt}", tag="sps")
                    for kt in range(CT):
                        nc.tensor.matmul(
                            psq[:],
                            lhsT=wqk[:, kt, mt * P:(mt + 1) * P],
                            rhs=pooled[:, kt],
                            start=(kt == 0), stop=(kt == CT - 1))
                    nc.scalar.activation(out=qk[:, mt], in_=psq[:],
                                         func=ACTF.Identity, bias=bqk[:, mt:mt + 1],
                                         scale=1.0)

                # ---- stage D: kg1 + gelu ----
                hsb = spool.tile([DQ + 1, 9], F32, name=f"h_{b}", tag="h")
                psh = sppool.tile([DQ, 9], F32, name=f"hps_{b}", tag="sps")
                for kt in range(CT):
                    nc.tensor.matmul(
                        psh[:],
                        lhsT=wkg1[:, kt, :],
                        rhs=qk[:, kt],
                        start=(kt == 0), stop=(kt == CT - 1))
                nc.scalar.activation(out=hsb[:DQ, :], in_=psh[:], func=ACTF.Gelu,
                                     bias=bkg1[:, 0:1], scale=1.0)
                nc.gpsimd.memset(hsb[DQ:DQ + 1, :], 1.0)  # bias row for kg2

                # ---- stage E: kg2 + mean subtraction -> k_sb ----
                ksb = spool.tile([P, CT, 9], F32, name=f"k_{b}", tag="ksb")
                ksum = spool.tile([P, CT], F32, name=f"ksum_{b}", tag="ksum")
                for mt in range(CT):
                    psk = sppool.tile([P, 9], F32, name=f"kps_{b}_{mt}", tag="sps")
                    nc.tensor.matmul(
                        psk[:],
                        lhsT=wkg2e[:, mt * P:(mt + 1) * P],
                        rhs=hsb[:],
                        start=True, stop=True)
                    nc.vector.tensor_reduce(out=ksum[:, mt:mt + 1], in_=psk[:],
                                            axis=AX.X, op=ALU.add)
                    nc.vector.tensor_scalar(
                        out=ksum[:, mt:mt + 1], in0=ksum[:, mt:mt + 1],
                        scalar1=fac9[:, mt:mt + 1], scalar2=None, op0=ALU.mult)
                    nc.vector.tensor_scalar(
                        out=ksb[:, mt], in0=psk[:],
                        scalar1=ksum[:, mt:mt + 1], scalar2=None, op0=ALU.subtract)

                # ---- stage F: diag(k) for PE taps ----
                kdiag = spool.tile([P, CT, len(PE_TAPS), P], BF16,
                                   name=f"kd_{b}", tag="kdiag")
                for ct in range(CT):
                    for ti, t in enumerate(PE_TAPS):
                        nc.vector.tensor_scalar(
                            out=kdiag[:, ct, ti], in0=eye[:],
                            scalar1=ksb[:, ct, t:t + 1], scalar2=None, op0=ALU.mult)

                # ---- stage G: depthwise ----
                # B copy: vpb[n] = vpad[n+1] (gpsimd, line-rate 1-input)
                for ct in range(CT):
                    nc.gpsimd.tensor_copy(vpbs[ct][:, 2:2 + PW * PW],
                                          vpads[ct][:, 3:3 + PW * PW])
                # padded accumulator; pads hold garbage, never read downstream
                dw = dwpool.tile([P, CT, PW * PW], BF16, name=f"dw_{b}", tag="dw")
                for ct in range(CT):
                    vpv = vpads[ct][:, 2:2 + PW * PW].rearrange(
                        "p (h w) -> p h w", h=PW)
                    # PE taps accumulate in psum; ACT copies into padded acc
                    for g in range(3):
                        dps = mmpool.tile([P, 2, 512], F32, name=f"dps_{b}_{ct}_{g}",
                                          tag="mm")
                        for s2 in range(2):
                            ch = g * 2 + s2
                            for ti, t in enumerate(PE_TAPS):
                                i, j = divmod(t, 3)
                                nc.tensor.matmul(
                                    dps[:, s2, :CH],
                                    lhsT=kdiag[:, ct, ti],
                                    rhs=vpv[:, ch * 9 + i: ch * 9 + i + 9, j:j + 54],
                                    start=(ti == 0), stop=(ti == len(PE_TAPS) - 1))
                        nc.scalar.activation(
                            out=dw[:, ct].rearrange("p (h w) -> p h w", h=PW)[
                                :, 1 + g * 18:1 + (g + 1) * 18, 1:55],
                            in_=dps[:, :, :CH],
                            func=ACTF.Copy, bias=0.0, scale=1.0)
                    # DVE taps: contiguous flat-shift multiply-add, bf16 2x
                    for t in DVE_TAPS:
                        i, j = divmod(t, 3)
                        off = (i - 1) * PW + (j - 1)
                        a0 = max(0, -off)
                        a0 -= a0 & 1  # round down to even (extends into pad cells)
                        ln = PW * PW - a0
                        if off % 2 == 0:
                            src = vpads[ct][:, 2 + a0 + off: 2 + a0 + off + ln]
                        else:
                            src = vpbs[ct][:, 2 + a0 + off - 1: 2 + a0 + off - 1 + ln]
                        nc.vector.scalar_tensor_tensor(
                            out=dw[:, ct, a0:a0 + ln], in0=src,
                            scalar=ksb[:, ct, t:t + 1], in1=dw[:, ct, a0:a0 + ln],
                            op0=ALU.mult, op1=ALU.add)

                # ---- stage H: proj conv + bias -> y ----
                for mt in range(CT):
                    for g in range(3):
                        ps = mmpool.tile([P, 2, 512], F32, name=f"pps_{b}_{mt}_{g}",
                                         tag="mm")
                        for s in range(2):
                            ch = g * 2 + s
                            for kt in range(CT):
                                nc.tensor.matmul(
                                    ps[:, s, :CH],
                                    lhsT=wproj[:, kt, mt * P:(mt + 1) * P],
                                    rhs=dw[:, kt].rearrange(
                                        "p (h w) -> p h w", h=PW)[
                                        :, ch * 9 + 1: ch * 9 + 10, 1:55],
                                    start=(kt == 0), stop=(kt == CT - 1))
                        ych = ypool.tile([P, BIG], F32, name=f"y_{b}_{mt}_{g}",
                                         tag="ych")
                        nc.scalar.activation(out=ych[:], in_=ps[:, :, :CH],
                                             func=ACTF.Identity,
                                             bias=bproj[:, mt:mt + 1], scale=1.0)
                        nc.sync.dma_start(
                            y_d[b, mt, :, g * BIG:(g + 1) * BIG], ych[:])
    nc.compile()
    return nc


def _prep_inputs(x, w_qk, b_qk, w_kg1, b_kg1, w_kg2, b_kg2, w_v, b_v,
                 w_proj, b_proj, beta):
    bf = ml_dtypes.bfloat16
    f32 = np.float32

    def lay_w(w, dt):  # (O, Cin) -> lhsT layout [p, kt, O]
        wt = np.ascontiguousarray(w.T.reshape(CT, P, -1).transpose(1, 0, 2))
        return wt.astype(dt)

    def lay_b(v):  # (C,) -> [p, ct]
        return np.ascontiguousarray(v.reshape(CT, P).T).astype(f32)

    consts = {
        "wv": lay_w(w_v, bf),
        "wproj": lay_w(w_proj, bf),
        "wqk": lay_w(w_qk / 324.0, f32),
        "wkg1": lay_w(w_kg1, f32),
        "wkg2e": np.ascontiguousarray(
            np.vstack([w_kg2.T, b_kg2[None, :]])).astype(f32),
        "bv": lay_b(b_v),
        "bqk": lay_b(b_qk),
        "bkg1": np.ascontiguousarray(b_kg1.reshape(DQ, 1)).astype(f32),
        "bproj": lay_b(b_proj),
        "fac9": lay_b(1.0 / (1.0 + np.exp(-beta.astype(np.float64))) / 9.0),
        "eye": np.eye(P, dtype=bf),
    }
    xs = np.ascontiguousarray(
        x.reshape(8, B_LOC, CT, P, HW)).astype(bf)
    in_maps = [dict(consts, x=np.ascontiguousarray(xs[c])) for c in range(8)]
    return in_maps


_CACHED_NC = None


def kernel(**inputs):
    global _CACHED_NC
    in_maps = _prep_inputs(**{k: np.asarray(v) for k, v in inputs.items()})
    if _CACHED_NC is None:
        _CACHED_NC = build_program()
    res = run_bass_kernel_spmd(_CACHED_NC, in_maps, core_ids=list(range(8)))
    ys = np.stack([r["y"] for r in res.results])  # (8, 4, 3, 128, 2916)
    return ys.reshape(32, 384, 54, 54).astype(np.float32)



# revision 8
# speedup vs baseline: 1.8091x; 1.8091x over previous
"""Trainium2 Bass kernel for nn_ConvocationV3 (dense_cnn).

Pipeline per sample (B=32, C=384, H=W=54, K=3):
  value = conv1x1(x, w_v) ; qk = pool3x3(conv1x1(x, w_qk)) = conv1x1(pool3x3(x), w_qk)
  h = gelu(conv1x1(qk, w_kg1)) ; kernels = conv1x1(h, w_kg2)
  kernels -= sigmoid(beta)/9 * sum_taps(kernels)
  out = depthwise3x3(value, kernels)  (per-sample, per-channel kernels)
  y = conv1x1(out, w_proj)

v2: depthwise runs entirely on the tensor engine as fp8 DoubleRow diag
matmuls (2 taps per pass): value plane stored as fp8*SV in a padded
56x56 layout; tap pairs (sorted by flat offset) are contracted via
lhsT = [diag(k_t0), diag(k_t1)] with a custom rhs access pattern
[p][2: stride d1-d0][cols: stride 1].  Convs stay bf16.  proj of
sample b-1 is emitted between value(b) and dw(b) so the PE stays busy
while the small kernel-generation chain settles.

Sharding: data-parallel over batch, 4 samples per core on 8 cores.
"""

import numpy as np
import ml_dtypes

import concourse.bass as bass
import concourse.bacc as bacc
import concourse.mybir as mybir
import concourse.tile as tile
from concourse.ap import AP
from concourse.bass_utils import run_bass_kernel_spmd

F32 = mybir.dt.float32
BF16 = mybir.dt.bfloat16
F8 = mybir.dt.float8e4
AX = mybir.AxisListType
ALU = mybir.AluOpType
ACTF = mybir.ActivationFunctionType
DR = mybir.MatmulPerfMode.DoubleRow

B_LOC = 4          # samples per core
CT = 3             # channel tiles (384 = 3*128)
P = 128
HW = 2916          # 54*54
PW = 56            # padded width/height
BIG = 972          # dma/act chunk (18 rows of 54)
CH = 486           # matmul free chunk (9 rows of 54)
DQ = 96
LEAD = 64          # lead slack in the padded value plane
VP_N = LEAD + PW * PW + 64

SV = 16.0          # fp8 scale for the value plane
SK = 512.0         # fp8 scale for the dynamic kernels (folded into wkg2e)

# taps t = 3*i + j sorted by flat offset (i-1)*56 + (j-1):
# deltas -57,-56,-55,-1,0,+1,+55,+56,+57.  DoubleRow pairs:
# (t0,t1),(t2,t3),(t4,t5),(t6,t7),(t8,zero) with strides 1,54,1,1,1.
PAIR_D0 = [-57, -55, 0, 55, 57]
PAIR_DP = [1, 54, 1, 1, 1]


def _pair_ap(t_ap, base, delta, cols):
    """[p][2: stride delta][cols: stride 1] view over a flat [P, N] tile."""
    return AP(t_ap.tensor, t_ap.offset + base,
              [list(t_ap.ap[0]), [delta, 2], [1, cols]])


def build_program():
    nc = bacc.Bacc("TRN2", target_bir_lowering=False, debug=False)

    x_d = nc.dram_tensor("x", [B_LOC, CT, P, HW], BF16, kind="ExternalInput").ap()
    wv_d = nc.dram_tensor("wv", [P, CT, 384], BF16, kind="ExternalInput").ap()
    wproj_d = nc.dram_tensor("wproj", [P, CT, 384], BF16, kind="ExternalInput").ap()
    wqk_d = nc.dram_tensor("wqk", [P, CT, 384], F32, kind="ExternalInput").ap()
    wkg1_d = nc.dram_tensor("wkg1", [P, CT, DQ], F32, kind="ExternalInput").ap()
    wkg2e_d = nc.dram_tensor("wkg2e", [DQ + 1, 384], F32, kind="ExternalInput").ap()
    bv_d = nc.dram_tensor("bv", [P, CT], F32, kind="ExternalInput").ap()
    bvs_d = nc.dram_tensor("bvs", [P, CT], F32, kind="ExternalInput").ap()
    bqk_d = nc.dram_tensor("bqk", [P, CT], F32, kind="ExternalInput").ap()
    bkg1_d = nc.dram_tensor("bkg1", [DQ, 1], F32, kind="ExternalInput").ap()
    bproj_d = nc.dram_tensor("bproj", [P, CT], F32, kind="ExternalInput").ap()
    fac9_d = nc.dram_tensor("fac9", [P, CT], F32, kind="ExternalInput").ap()
    eye_d = nc.dram_tensor("eye", [P, P], BF16, kind="ExternalInput").ap()

    y_d = nc.dram_tensor("y", [B_LOC, CT, P, HW], BF16, kind="ExternalOutput").ap()

    with tile.TileContext(nc) as tc:
        with (
            tc.tile_pool(name="const", bufs=1) as cpool,
            tc.tile_pool(name="xch", bufs=4) as xpool,
            tc.tile_pool(name="vpad", bufs=2) as vppool,
            tc.tile_pool(name="dw", bufs=2) as dwpool,
            tc.tile_pool(name="ych", bufs=4) as ypool,
            tc.tile_pool(name="small", bufs=2) as spool,
            tc.tile_pool(name="mm", bufs=3, space="PSUM") as mmpool,
            tc.tile_pool(name="smallps", bufs=2, space="PSUM") as sppool,
        ):
            # ---- constants ----
            wv = cpool.tile([P, CT, 384], BF16, name="wv_sb")
            wproj = cpool.tile([P, CT, 384], BF16, name="wproj_sb")
            wqk = cpool.tile([P, CT, 384], F32, name="wqk_sb")
            wkg1 = cpool.tile([P, CT, DQ], F32, name="wkg1_sb")
            wkg2e = cpool.tile([DQ + 1, 384], F32, name="wkg2e_sb")
            bv = cpool.tile([P, CT], F32, name="bv_sb")
            bvs = cpool.tile([P, CT], F32, name="bvs_sb")
            bqk = cpool.tile([P, CT], F32, name="bqk_sb")
            bkg1 = cpool.tile([DQ, 1], F32, name="bkg1_sb")
            bproj = cpool.tile([P, CT], F32, name="bproj_sb")
            fac9 = cpool.tile([P, CT], F32, name="fac9_sb")
            eye = cpool.tile([P, P], BF16, name="eye_sb")
            for t_sb, t_dr in [(wv, wv_d), (wproj, wproj_d), (wqk, wqk_d),
                               (wkg1, wkg1_d), (wkg2e, wkg2e_d), (bv, bv_d),
                               (bvs, bvs_d),
                               (bqk, bqk_d), (bkg1, bkg1_d), (bproj, bproj_d),
                               (fac9, fac9_d), (eye, eye_d)]:
                nc.sync.dma_start(t_sb[:], t_dr[:])

            prev = None  # (dw tile, b) pending proj

            def emit_proj(dwt, b):
                for mt in range(CT):
                    for g in range(3):
                        ps = mmpool.tile([P, 2, 512], F32, name=f"pps_{b}_{mt}_{g}",
                                         tag="mm")
                        for s in range(2):
                            ch = g * 2 + s
                            for kt in range(CT):
                                nc.tensor.matmul(
                                    ps[:, s, :CH],
                                    lhsT=wproj[:, kt, mt * P:(mt + 1) * P],
                                    rhs=dwt[:, kt].rearrange(
                                        "p (h w) -> p h w", h=PW)[
                                        :, ch * 9 + 1: ch * 9 + 10, 1:55],
                                    start=(kt == 0), stop=(kt == CT - 1))
                        ych = ypool.tile([P, BIG], BF16, name=f"y_{b}_{mt}_{g}",
                                         tag="ych")
                        if mt == 2:
                            nc.scalar.activation(out=ych[:], in_=ps[:, :, :CH],
                                                 func=ACTF.Identity,
                                                 bias=bproj[:, mt:mt + 1], scale=1.0)
                        else:
                            nc.vector.tensor_scalar(
                                out=ych[:], in0=ps[:, :, :CH],
                                scalar1=bproj[:, mt:mt + 1], scalar2=None,
                                op0=ALU.add)
                        nc.sync.dma_start(
                            y_d[b, mt, :, g * BIG:(g + 1) * BIG], ych[:])

            for b in range(B_LOC):
                # ---- stage A: x load, pooling stage 1, value conv -> vpad ----
                pool1 = spool.tile([P, CT, 54, 3], F32, name=f"pool1_{b}", tag="pool1")
                vpads = []
                vpvs = []
                for ct in range(CT):
                    vp = vppool.tile([P, VP_N], F8, name=f"vpad_{b}_{ct}",
                                     tag=f"vpad{ct}")
                    vpads.append(vp)
                    vpv = vp[:, LEAD:LEAD + PW * PW].rearrange(
                        "p (h w) -> p h w", h=PW)
                    vpvs.append(vpv)
                    # zero lead+top row / bottom row+tail, and side cols
                    nc.gpsimd.memset(vp[:, 0:LEAD + PW], 0.0)
                    nc.gpsimd.memset(vp[:, LEAD + PW * (PW - 1):VP_N], 0.0)
                    nc.gpsimd.memset(vpv[:, 1:PW - 1, 0:1], 0.0)
                    nc.gpsimd.memset(vpv[:, 1:PW - 1, PW - 1:PW], 0.0)

                for g in range(3):  # big chunks of 18 rows
                    xch = xpool.tile([P, CT, BIG], BF16, name=f"x_{b}_{g}", tag="xch")
                    nc.sync.dma_start(
                        xch[:], x_d[b, :, :, g * BIG:(g + 1) * BIG].transpose([1, 0, 2]))
                    for kt in range(CT):
                        nc.vector.tensor_reduce(
                            out=pool1[:, kt, g * 18:(g + 1) * 18, :],
                            in_=xch[:, kt].rearrange("p (h wb w) -> p h wb w", wb=3, w=18),
                            axis=AX.X, op=ALU.add)
                    for mt in range(CT):
                        ps = mmpool.tile([P, 2, 512], F32, name=f"vps_{b}_{g}_{mt}", tag="mm")
                        for s in range(2):
                            for kt in range(CT):
                                nc.tensor.matmul(
                                    ps[:, s, :CH],
                                    lhsT=wv[:, kt, mt * P:(mt + 1) * P],
                                    rhs=xch[:, kt, s * CH:(s + 1) * CH],
                                    start=(kt == 0), stop=(kt == CT - 1))
                        # vpad <- (value + bias) * SV, fp8
                        out_ap = vpvs[mt][:, 1 + g * 18:1 + (g + 1) * 18, 1:55]
                        if mt == 2:
                            # ACT: out = in*SV + bv*SV
                            nc.scalar.activation(
                                out=out_ap, in_=ps[:, :, :CH],
                                func=ACTF.Identity, bias=bvs[:, mt:mt + 1],
                                scale=SV)
                        else:
                            nc.vector.tensor_scalar(
                                out=out_ap, in0=ps[:, :, :CH],
                                scalar1=bv[:, mt:mt + 1], scalar2=SV,
                                op0=ALU.add, op1=ALU.mult)

                # ---- stage H for previous sample (keeps PE busy while the
                # small kernel-generation chain below settles) ----
                if prev is not None:
                    emit_proj(*prev)

                # ---- stage B: pooling stage 2 (sum over 324; /324 in wqk) ----
                pooled = spool.tile([P, CT, 9], F32, name=f"pooled_{b}", tag="pooled")
                for kt in range(CT):
                    nc.vector.tensor_reduce(
                        out=pooled[:, kt].rearrange("p (hb wb) -> p hb wb", hb=3),
                        in_=pool1[:, kt].rearrange("p (hb hs) wb -> p hb wb hs", hb=3),
                        axis=AX.X, op=ALU.add)

                # ---- stage C: qk conv (f32, tiny) ----
                qk = spool.tile([P, CT, 9], F32, name=f"qk_{b}", tag="qk")
                for mt in range(CT):
                    psq = sppool.tile([P, 9], F32, name=f"qps_{b}_{mt}", tag="sps")
                    for kt in range(CT):
                        nc.tensor.matmul(
                            psq[:],
                            lhsT=wqk[:, kt, mt * P:(mt + 1) * P],
                            rhs=pooled[:, kt],
                            start=(kt == 0), stop=(kt == CT - 1))
                    nc.scalar.activation(out=qk[:, mt], in_=psq[:],
                                         func=ACTF.Identity, bias=bqk[:, mt:mt + 1],
                                         scale=1.0)

                # ---- stage D: kg1 + gelu ----
                hsb = spool.tile([DQ + 1, 9], F32, name=f"h_{b}", tag="h")
                psh = sppool.tile([DQ, 9], F32, name=f"hps_{b}", tag="sps")
                for kt in range(CT):
                    nc.tensor.matmul(
                        psh[:],
                        lhsT=wkg1[:, kt, :],
                        rhs=qk[:, kt],
                        start=(kt == 0), stop=(kt == CT - 1))
                nc.scalar.activation(out=hsb[:DQ, :], in_=psh[:], func=ACTF.Gelu,
                                     bias=bkg1[:, 0:1], scale=1.0)
                nc.gpsimd.memset(hsb[DQ:DQ + 1, :], 1.0)  # bias row for kg2

                # ---- stage E: kg2 (scaled by SK) + mean subtraction -> ksb ----
                ksb = spool.tile([P, CT, 9], F32, name=f"k_{b}", tag="ksb")
                ksum = spool.tile([P, CT], F32, name=f"ksum_{b}", tag="ksum")
                for mt in range(CT):
                    psk = sppool.tile([P, 9], F32, name=f"kps_{b}_{mt}", tag="sps")
                    nc.tensor.matmul(
                        psk[:],
                        lhsT=wkg2e[:, mt * P:(mt + 1) * P],
                        rhs=hsb[:],
                        start=True, stop=True)
                    nc.vector.tensor_reduce(out=ksum[:, mt:mt + 1], in_=psk[:],
                                            axis=AX.X, op=ALU.add)
                    nc.vector.tensor_scalar(
                        out=ksum[:, mt:mt + 1], in0=ksum[:, mt:mt + 1],
                        scalar1=fac9[:, mt:mt + 1], scalar2=None, op0=ALU.mult)
                    nc.vector.tensor_scalar(
                        out=ksb[:, mt], in0=psk[:],
                        scalar1=ksum[:, mt:mt + 1], scalar2=None, op0=ALU.subtract)

                # ---- stage F: fp8 DoubleRow lhsT pairs [diag(k_t0)|diag(k_t1)] ----
                kd = spool.tile([P, CT, 5, 2, P], F8, name=f"kd_{b}", tag="kd")
                for ct in range(CT):
                    for t in range(9):
                        nc.scalar.activation(
                            out=kd[:, ct, t // 2, t % 2, :], in_=eye[:],
                            func=ACTF.Copy, bias=0.0,
                            scale=ksb[:, ct, t:t + 1])
                    nc.gpsimd.memset(kd[:, ct, 4, 1, :], 0.0)

                # ---- stage G: depthwise, all 9 taps on PE via fp8 DoubleRow ----
                dwt = dwpool.tile([P, CT, PW * PW], BF16, name=f"dw_{b}", tag="dw")
                for ct in range(CT):
                    vfull = vpads[ct][:]
                    # 6 full 486-col chunks in 3 psum tiles + 220-col tail
                    tiles = [mmpool.tile([P, 2, 512], F32,
                                         name=f"dps_{b}_{ct}_{i}", tag="mm")
                             for i in range(3)]
                    tailt = mmpool.tile([P, 2, 512], F32,
                                        name=f"dpt_{b}_{ct}", tag="mm")
                    for c in range(7):
                        cols = CH if c < 6 else PW * PW - 6 * CH
                        pst = tiles[c // 2][:, c % 2, :cols] if c < 6 \
                            else tailt[:, 0, :cols]
                        for pi in range(5):
                            nc.tensor.matmul(
                                pst,
                                lhsT=kd[:, ct, pi, :, :],
                                rhs=_pair_ap(vfull, LEAD + c * CH + PAIR_D0[pi],
                                             PAIR_DP[pi], cols),
                                start=(pi == 0), stop=(pi == 4),
                                perf_mode=DR)
                    # drain psum -> dw (bf16), scale 1/(SV*SK)
                    for i in range(3):
                        nc.scalar.activation(
                            out=dwt[:, ct, i * 2 * CH:(i + 1) * 2 * CH],
                            in_=tiles[i][:, :, :CH],
                            func=ACTF.Copy, bias=0.0, scale=1.0 / (SV * SK))
                    nc.scalar.activation(
                        out=dwt[:, ct, 6 * CH:PW * PW],
                        in_=tailt[:, 0, :PW * PW - 6 * CH],
                        func=ACTF.Copy, bias=0.0, scale=1.0 / (SV * SK))

                prev = (dwt, b)

            emit_proj(*prev)
    nc.compile()
    return nc


def _prep_inputs(x, w_qk, b_qk, w_kg1, b_kg1, w_kg2, b_kg2, w_v, b_v,
                 w_proj, b_proj, beta):
    bf = ml_dtypes.bfloat16
    f32 = np.float32

    def lay_w(w, dt):  # (O, Cin) -> lhsT layout [p, kt, O]
        wt = np.ascontiguousarray(w.T.reshape(CT, P, -1).transpose(1, 0, 2))
        return wt.astype(dt)

    def lay_b(v):  # (C,) -> [p, ct]
        return np.ascontiguousarray(v.reshape(CT, P).T).astype(f32)

    consts = {
        "wv": lay_w(w_v, bf),
        "wproj": lay_w(w_proj, bf),
        "wqk": lay_w(w_qk / 324.0, f32),
        "wkg1": lay_w(w_kg1, f32),
        "wkg2e": np.ascontiguousarray(
            np.vstack([w_kg2.T, b_kg2[None, :]]) * SK).astype(f32),
        "bv": lay_b(b_v),
        "bvs": lay_b(b_v * SV),
        "bqk": lay_b(b_qk),
        "bkg1": np.ascontiguousarray(b_kg1.reshape(DQ, 1)).astype(f32),
        "bproj": lay_b(b_proj),
        "fac9": lay_b(1.0 / (1.0 + np.exp(-beta.astype(np.float64))) / 9.0),
        "eye": np.eye(P, dtype=bf),
    }
    xs = np.ascontiguousarray(
        x.reshape(8, B_LOC, CT, P, HW)).astype(bf)
    in_maps = [dict(consts, x=np.ascontiguousarray(xs[c])) for c in range(8)]
    return in_maps


_CACHED_NC = None


def kernel(**inputs):
    global _CACHED_NC
    in_maps = _prep_inputs(**{k: np.asarray(v) for k, v in inputs.items()})
    if _CACHED_NC is None:
        _CACHED_NC = build_program()
    res = run_bass_kernel_spmd(_CACHED_NC, in_maps, core_ids=list(range(8)))
    ys = np.stack([np.asarray(r["y"]).astype(np.float32)
                   for r in res.results])  # (8, 4, 3, 128, 2916)
    return ys.reshape(32, 384, 54, 54)


# revision 10
# speedup vs baseline: 1.9816x; 1.0954x over previous
"""Trainium2 Bass kernel for nn_ConvocationV3 (dense_cnn).

Pipeline per sample (B=32, C=384, H=W=54, K=3):
  value = conv1x1(x, w_v) ; qk = pool3x3(conv1x1(x, w_qk)) = conv1x1(pool3x3(x), w_qk)
  h = gelu(conv1x1(qk, w_kg1)) ; kernels = conv1x1(h, w_kg2)
  kernels -= sigmoid(beta)/9 * sum_taps(kernels)
  out = depthwise3x3(value, kernels)  (per-sample, per-channel kernels)
  y = conv1x1(out, w_proj)

v3: depthwise runs entirely on the tensor engine as fp8 DoubleRow diag
matmuls (2 taps per pass): value plane stored as fp8*SV in a padded
56x56 layout; tap pairs (sorted by flat offset) are contracted via
lhsT = [diag(k_t0)|diag(k_t1)] with a custom rhs access pattern
[p][2: stride d1-d0][cols: stride 1].  The big convs stay bf16; the
tiny qk/kernel-gen path is bf16 (fp32 matmuls cost 2 half-rate passes).
Emission order per sample: x-load+pool, kernel-gen chain, value conv,
proj of the previous sample (keeps the PE busy while the chain
settles on DVE/Scalar), then the depthwise.

Sharding: data-parallel over batch, 4 samples per core on 8 cores.
"""

import numpy as np
import ml_dtypes

import concourse.bass as bass
import concourse.bacc as bacc
import concourse.mybir as mybir
import concourse.tile as tile
from concourse.ap import AP
from concourse.bass_utils import run_bass_kernel_spmd

F32 = mybir.dt.float32
BF16 = mybir.dt.bfloat16
F8 = mybir.dt.float8e4
AX = mybir.AxisListType
ALU = mybir.AluOpType
ACTF = mybir.ActivationFunctionType
DR = mybir.MatmulPerfMode.DoubleRow

B_LOC = 4          # samples per core
CT = 3             # channel tiles (384 = 3*128)
P = 128
HW = 2916          # 54*54
PW = 56            # padded width/height
BIG = 972          # dma/act chunk (18 rows of 54)
CH = 486           # matmul free chunk (9 rows of 54)
DQ = 96
LEAD = 64          # lead slack in the padded value plane
VP_N = LEAD + PW * PW + 64
TAIL = PW * PW - 6 * CH  # 220

SV = 16.0          # fp8 scale for the value plane
SK = 512.0         # fp8 scale for the dynamic kernels (folded into wkg2e)

# taps t = 3*i + j sorted by flat offset (i-1)*56 + (j-1):
# deltas -57,-56,-55,-1,0,+1,+55,+56,+57.  DoubleRow pairs:
# (t0,t1),(t2,t3),(t4,t5),(t6,t7),(t8,zero) with strides 1,54,1,1,1.
PAIR_D0 = [-57, -55, 0, 55, 57]
PAIR_DP = [1, 54, 1, 1, 1]


def _pair_ap(t_ap, base, delta, cols):
    """[p][2: stride delta][cols: stride 1] view over a flat [P, N] tile."""
    return AP(t_ap.tensor, t_ap.offset + base,
              [list(t_ap.ap[0]), [delta, 2], [1, cols]])


def build_program():
    nc = bacc.Bacc("TRN2", target_bir_lowering=False, debug=False)

    x_d = nc.dram_tensor("x", [B_LOC, CT, P, HW], BF16, kind="ExternalInput").ap()
    wv_d = nc.dram_tensor("wv", [P, CT, 384], BF16, kind="ExternalInput").ap()
    wproj_d = nc.dram_tensor("wproj", [P, CT, 384], BF16, kind="ExternalInput").ap()
    wqk_d = nc.dram_tensor("wqk", [P, CT, 384], BF16, kind="ExternalInput").ap()
    wkg1_d = nc.dram_tensor("wkg1", [P, CT, DQ], BF16, kind="ExternalInput").ap()
    wkg2e_d = nc.dram_tensor("wkg2e", [DQ + 1, 384], BF16, kind="ExternalInput").ap()
    bv_d = nc.dram_tensor("bv", [P, CT], F32, kind="ExternalInput").ap()
    bvs_d = nc.dram_tensor("bvs", [P, CT], F32, kind="ExternalInput").ap()
    bqk_d = nc.dram_tensor("bqk", [P, CT], F32, kind="ExternalInput").ap()
    bkg1_d = nc.dram_tensor("bkg1", [DQ, 1], F32, kind="ExternalInput").ap()
    bproj_d = nc.dram_tensor("bproj", [P, CT], F32, kind="ExternalInput").ap()
    fac9_d = nc.dram_tensor("fac9", [P, CT], F32, kind="ExternalInput").ap()
    eye_d = nc.dram_tensor("eye", [P, P], BF16, kind="ExternalInput").ap()

    y_d = nc.dram_tensor("y", [B_LOC, CT, P, HW], BF16, kind="ExternalOutput").ap()

    with tile.TileContext(nc) as tc, nc.allow_low_precision(
            reason="bf16 qk path: pooled sums round once; error ~0.4% on a "
                   "path whose fp8 kernel quantization dominates"):
        with (
            tc.tile_pool(name="const", bufs=1) as cpool,
            tc.tile_pool(name="xch", bufs=4) as xpool,
            tc.tile_pool(name="vpad", bufs=2) as vppool,
            tc.tile_pool(name="dw", bufs=2) as dwpool,
            tc.tile_pool(name="ych", bufs=4) as ypool,
            tc.tile_pool(name="small", bufs=2) as spool,
            tc.tile_pool(name="mm", bufs=3, space="PSUM") as mmpool,
            tc.tile_pool(name="smallps", bufs=2, space="PSUM") as sppool,
        ):
            # ---- constants ----
            wv = cpool.tile([P, CT, 384], BF16, name="wv_sb")
            wproj = cpool.tile([P, CT, 384], BF16, name="wproj_sb")
            wqk = cpool.tile([P, CT, 384], BF16, name="wqk_sb")
            wkg1 = cpool.tile([P, CT, DQ], BF16, name="wkg1_sb")
            wkg2e = cpool.tile([DQ + 1, 384], BF16, name="wkg2e_sb")
            bv = cpool.tile([P, CT], F32, name="bv_sb")
            bvs = cpool.tile([P, CT], F32, name="bvs_sb")
            bqk = cpool.tile([P, CT], F32, name="bqk_sb")
            bkg1 = cpool.tile([DQ, 1], F32, name="bkg1_sb")
            bproj = cpool.tile([P, CT], F32, name="bproj_sb")
            fac9 = cpool.tile([P, CT], F32, name="fac9_sb")
            eye = cpool.tile([P, P], BF16, name="eye_sb")
            for t_sb, t_dr in [(wv, wv_d), (wproj, wproj_d), (wqk, wqk_d),
                               (wkg1, wkg1_d), (wkg2e, wkg2e_d), (bv, bv_d),
                               (bvs, bvs_d),
                               (bqk, bqk_d), (bkg1, bkg1_d), (bproj, bproj_d),
                               (fac9, fac9_d), (eye, eye_d)]:
                nc.sync.dma_start(t_sb[:], t_dr[:])

            prev = None  # (dw tile, b) pending proj

            def emit_proj(dwt, b):
                for mt in range(CT):
                    for g in range(3):
                        ps = mmpool.tile([P, 2, 512], F32, name=f"pps_{b}_{mt}_{g}",
                                         tag="mm")
                        for s in range(2):
                            ch = g * 2 + s
                            for kt in range(CT):
                                nc.tensor.matmul(
                                    ps[:, s, :CH],
                                    lhsT=wproj[:, kt, mt * P:(mt + 1) * P],
                                    rhs=dwt[:, kt].rearrange(
                                        "p (h w) -> p h w", h=PW)[
                                        :, ch * 9 + 1: ch * 9 + 10, 1:55],
                                    start=(kt == 0), stop=(kt == CT - 1))
                        ych = ypool.tile([P, BIG], BF16, name=f"y_{b}_{mt}_{g}",
                                         tag="ych")
                        if mt == 2:
                            nc.scalar.activation(out=ych[:], in_=ps[:, :, :CH],
                                                 func=ACTF.Identity,
                                                 bias=bproj[:, mt:mt + 1], scale=1.0)
                        else:
                            nc.vector.tensor_scalar(
                                out=ych[:], in0=ps[:, :, :CH],
                                scalar1=bproj[:, mt:mt + 1], scalar2=None,
                                op0=ALU.add)
                        nc.sync.dma_start(
                            y_d[b, mt, :, g * BIG:(g + 1) * BIG], ych[:])

            for b in range(B_LOC):
                # ---- stage A1: x load + pooling stage 1 ----
                pool1 = spool.tile([P, CT, 54, 3], F32, name=f"pool1_{b}", tag="pool1")
                xchs = []
                for g in range(3):  # big chunks of 18 rows
                    xch = xpool.tile([P, CT, BIG], BF16, name=f"x_{b}_{g}", tag="xch")
                    xchs.append(xch)
                    nc.sync.dma_start(
                        xch[:], x_d[b, :, :, g * BIG:(g + 1) * BIG].transpose([1, 0, 2]))
                    for kt in range(CT):
                        nc.vector.tensor_reduce(
                            out=pool1[:, kt, g * 18:(g + 1) * 18, :],
                            in_=xch[:, kt].rearrange("p (h wb w) -> p h wb w", wb=3, w=18),
                            axis=AX.X, op=ALU.add)

                # ---- stage A2: value conv -> vpad (fp8 * SV) ----
                vpads = []
                vpvs = []
                for ct in range(CT):
                    vp = vppool.tile([P, VP_N], F8, name=f"vpad_{b}_{ct}",
                                     tag=f"vpad{ct}")
                    vpads.append(vp)
                    vpv = vp[:, LEAD:LEAD + PW * PW].rearrange(
                        "p (h w) -> p h w", h=PW)
                    vpvs.append(vpv)
                    # zero lead+top row / bottom row+tail, and side cols
                    nc.gpsimd.memset(vp[:, 0:LEAD + PW], 0.0)
                    nc.gpsimd.memset(vp[:, LEAD + PW * (PW - 1):VP_N], 0.0)
                    nc.gpsimd.memset(vpv[:, 1:PW - 1, 0:1], 0.0)
                    nc.gpsimd.memset(vpv[:, 1:PW - 1, PW - 1:PW], 0.0)

                for g in range(3):
                    for mt in range(CT):
                        ps = mmpool.tile([P, 2, 512], F32, name=f"vps_{b}_{g}_{mt}", tag="mm")
                        for s in range(2):
                            for kt in range(CT):
                                nc.tensor.matmul(
                                    ps[:, s, :CH],
                                    lhsT=wv[:, kt, mt * P:(mt + 1) * P],
                                    rhs=xchs[g][:, kt, s * CH:(s + 1) * CH],
                                    start=(kt == 0), stop=(kt == CT - 1))
                        # vpad <- (value + bias) * SV, fp8
                        out_ap = vpvs[mt][:, 1 + g * 18:1 + (g + 1) * 18, 1:55]
                        if mt == 2:
                            # ACT: out = in*SV + bv*SV
                            nc.scalar.activation(
                                out=out_ap, in_=ps[:, :, :CH],
                                func=ACTF.Identity, bias=bvs[:, mt:mt + 1],
                                scale=SV)
                        else:
                            nc.vector.tensor_scalar(
                                out=out_ap, in0=ps[:, :, :CH],
                                scalar1=bv[:, mt:mt + 1], scalar2=SV,
                                op0=ALU.add, op1=ALU.mult)

                # ---- stage B: pooling stage 2 (sum over 324; /324 in wqk) ----
                pooled = spool.tile([P, CT, 9], BF16, name=f"pooled_{b}", tag="pooled")
                for kt in range(CT):
                    nc.vector.tensor_reduce(
                        out=pooled[:, kt].rearrange("p (hb wb) -> p hb wb", hb=3),
                        in_=pool1[:, kt].rearrange("p (hb hs) wb -> p hb wb hs", hb=3),
                        axis=AX.X, op=ALU.add)

                # ---- stage C: qk conv (bf16, tiny) ----
                qk = spool.tile([P, CT, 9], BF16, name=f"qk_{b}", tag="qk")
                for mt in range(CT):
                    psq = sppool.tile([P, 9], F32, name=f"qps_{b}_{mt}", tag="sps")
                    for kt in range(CT):
                        nc.tensor.matmul(
                            psq[:],
                            lhsT=wqk[:, kt, mt * P:(mt + 1) * P],
                            rhs=pooled[:, kt],
                            start=(kt == 0), stop=(kt == CT - 1))
                    nc.scalar.activation(out=qk[:, mt], in_=psq[:],
                                         func=ACTF.Identity, bias=bqk[:, mt:mt + 1],
                                         scale=1.0)

                # ---- stage D: kg1 + gelu ----
                hsb = spool.tile([DQ + 1, 9], BF16, name=f"h_{b}", tag="h")
                psh = sppool.tile([DQ, 9], F32, name=f"hps_{b}", tag="sps")
                for kt in range(CT):
                    nc.tensor.matmul(
                        psh[:],
                        lhsT=wkg1[:, kt, :],
                        rhs=qk[:, kt],
                        start=(kt == 0), stop=(kt == CT - 1))
                nc.scalar.activation(out=hsb[:DQ, :], in_=psh[:], func=ACTF.Gelu,
                                     bias=bkg1[:, 0:1], scale=1.0)
                nc.gpsimd.memset(hsb[DQ:DQ + 1, :], 1.0)  # bias row for kg2

                # ---- stage E: kg2 (scaled by SK) + mean subtraction -> ksb ----
                ksb = spool.tile([P, CT, 9], F32, name=f"k_{b}", tag="ksb")
                ksum = spool.tile([P, CT], F32, name=f"ksum_{b}", tag="ksum")
                for mt in range(CT):
                    psk = sppool.tile([P, 9], F32, name=f"kps_{b}_{mt}", tag="sps")
                    nc.tensor.matmul(
                        psk[:],
                        lhsT=wkg2e[:, mt * P:(mt + 1) * P],
                        rhs=hsb[:],
                        start=True, stop=True)
                    nc.vector.tensor_reduce(out=ksum[:, mt:mt + 1], in_=psk[:],
                                            axis=AX.X, op=ALU.add)
                    nc.vector.tensor_scalar(
                        out=ksum[:, mt:mt + 1], in0=ksum[:, mt:mt + 1],
                        scalar1=fac9[:, mt:mt + 1], scalar2=None, op0=ALU.mult)
                    nc.vector.tensor_scalar(
                        out=ksb[:, mt], in0=psk[:],
                        scalar1=ksum[:, mt:mt + 1], scalar2=None, op0=ALU.subtract)

                # ---- stage F: fp8 DoubleRow lhsT pairs [diag(k_t0)|diag(k_t1)] ----
                kd = spool.tile([P, CT, 5, 2, P], F8, name=f"kd_{b}", tag="kd")
                for ct in range(CT):
                    for t in range(9):
                        nc.scalar.activation(
                            out=kd[:, ct, t // 2, t % 2, :], in_=eye[:],
                            func=ACTF.Copy, bias=0.0,
                            scale=ksb[:, ct, t:t + 1])
                    nc.gpsimd.memset(kd[:, ct, 4, 1, :], 0.0)

                # ---- stage H for previous sample (keeps PE busy while the
                # kernel-gen chain for this sample settles) ----
                if prev is not None:
                    emit_proj(*prev)

                # ---- stage G: depthwise, all 9 taps on PE via fp8 DoubleRow ----
                dwt = dwpool.tile([P, CT, PW * PW], BF16, name=f"dw_{b}", tag="dw")
                for ct in range(CT):
                    vfull = vpads[ct][:]
                    # 6 full 486-col chunks in 3 psum tiles + 220-col tail
                    tiles = [mmpool.tile([P, 2, 512], F32,
                                         name=f"dps_{b}_{ct}_{i}", tag="mm")
                             for i in range(3)]
                    tailt = sppool.tile([P, 512], F32, name=f"dpt_{b}_{ct}",
                                        tag="sps")
                    for c in range(7):
                        cols = CH if c < 6 else TAIL
                        pst = tiles[c // 2][:, c % 2, :cols] if c < 6 \
                            else tailt[:, :cols]
                        for pi in range(5):
                            nc.tensor.matmul(
                                pst,
                                lhsT=kd[:, ct, pi, :, :],
                                rhs=_pair_ap(vfull, LEAD + c * CH + PAIR_D0[pi],
                                             PAIR_DP[pi], cols),
                                start=(pi == 0), stop=(pi == 4),
                                perf_mode=DR)
                    # drain psum -> dw (bf16), scale 1/(SV*SK); alternate
                    # engines so the drain latency halves at sample tails
                    for i in range(3):
                        dst = dwt[:, ct, i * 2 * CH:(i + 1) * 2 * CH]
                        if i == 1:
                            nc.vector.tensor_scalar(
                                out=dst, in0=tiles[i][:, :, :CH],
                                scalar1=1.0 / (SV * SK), scalar2=None,
                                op0=ALU.mult)
                        else:
                            nc.scalar.activation(
                                out=dst, in_=tiles[i][:, :, :CH],
                                func=ACTF.Copy, bias=0.0, scale=1.0 / (SV * SK))
                    nc.vector.tensor_scalar(
                        out=dwt[:, ct, 6 * CH:PW * PW], in0=tailt[:, :TAIL],
                        scalar1=1.0 / (SV * SK), scalar2=None, op0=ALU.mult)

                prev = (dwt, b)

            emit_proj(*prev)
    nc.compile()
    return nc


def _prep_inputs(x, w_qk, b_qk, w_kg1, b_kg1, w_kg2, b_kg2, w_v, b_v,
                 w_proj, b_proj, beta):
    bf = ml_dtypes.bfloat16
    f32 = np.float32

    def lay_w(w, dt):  # (O, Cin) -> lhsT layout [p, kt, O]
        wt = np.ascontiguousarray(w.T.reshape(CT, P, -1).transpose(1, 0, 2))
        return wt.astype(dt)

    def lay_b(v):  # (C,) -> [p, ct]
        return np.ascontiguousarray(v.reshape(CT, P).T).astype(f32)

    consts = {
        "wv": lay_w(w_v, bf),
        "wproj": lay_w(w_proj, bf),
        "wqk": lay_w(w_qk / 324.0, bf),
        "wkg1": lay_w(w_kg1, bf),
        "wkg2e": np.ascontiguousarray(
            np.vstack([w_kg2.T, b_kg2[None, :]]) * SK).astype(bf),
        "bv": lay_b(b_v),
        "bvs": lay_b(b_v * SV),
        "bqk": lay_b(b_qk),
        "bkg1": np.ascontiguousarray(b_kg1.reshape(DQ, 1)).astype(f32),
        "bproj": lay_b(b_proj),
        "fac9": lay_b(1.0 / (1.0 + np.exp(-beta.astype(np.float64))) / 9.0),
        "eye": np.eye(P, dtype=bf),
    }
    xs = np.ascontiguousarray(
        x.reshape(8, B_LOC, CT, P, HW)).astype(bf)
    in_maps = [dict(consts, x=np.ascontiguousarray(xs[c])) for c in range(8)]
    return in_maps


_CACHED_NC = None


def kernel(**inputs):
    global _CACHED_NC
    in_maps = _prep_inputs(**{k: np.asarray(v) for k, v in inputs.items()})
    if _CACHED_NC is None:
        _CACHED_NC = build_program()
    res = run_bass_kernel_spmd(_CACHED_NC, in_maps, core_ids=list(range(8)))
    ys = np.stack([np.asarray(r["y"]).astype(np.float32)
                   for r in res.results])  # (8, 4, 3, 128, 2916)
    return ys.reshape(32, 384, 54, 54)
